# revision 9
# baseline (speedup 1.0000x reference)
"""Trainium2 Bass kernel for nn_MessagePassingFlow (GNN message passing).

Strategy (8 NeuronCores, SPMD, no collectives):
  - Nodes are sharded contiguously: core k owns nodes [k*N/8, (k+1)*N/8).
  - Edges are sharded by DESTINATION node (row), so each core's segment-sum
    targets only its own node slice -> no all-reduce needed.
  - Per core, local nodes are packed into blocks of 128 nodes, balanced by
    degree (greedy), and edges laid out into fixed slots
    (nblk blocks x T tiles x 128 slots) sorted by node within a block.
  - Scatter-add is a one-hot matmul per tile into a per-block PSUM accumulator.
  - Edge invariants: V rows for the col endpoint come from the gpsimd
    dma_gather custom op reading per-half-core compacted tables (int16 idx);
    V rows + norms for the row endpoint are expanded from the block's 128
    nodes with a one-hot matmul.
  - Matmuls run in bf16 (fp32 PSUM accumulation); LayerNorm statistics and
    residual adds stay fp32.
"""

import os
import sys
import numpy as np

sys.path.insert(0, "/opt/trn_rl_repo")

import ml_dtypes

BF16 = ml_dtypes.bfloat16

D = 128
W = 16
AX = 3
NC_CORES = 8
BLK = 128
LN_EPS = 1e-5
COS_EPS = 1e-8

_PROGRAM_CACHE = {}
LAST_RESULTS = None
LAST_EXEC_NS = None


# ----------------------------------------------------------------------------
# Host-side sharding / slotting
# ----------------------------------------------------------------------------

def _balance_blocks(deg, nblk):
    """Greedy: assign nodes to blocks (<=128 nodes each), balancing edge load.
    Returns (blk_of, off_of, node_new)."""
    n = len(deg)
    order = np.argsort(-deg, kind="stable")
    loads = np.zeros(nblk, dtype=np.int64)
    used = np.zeros(nblk, dtype=np.int64)
    node_new = np.full(nblk * BLK, -1, dtype=np.int64)
    blk_of = np.empty(n, dtype=np.int64)
    off_of = np.empty(n, dtype=np.int64)
    for nd in order:
        b = int(np.argmin(np.where(used < BLK, loads, 1 << 60)))
        blk_of[nd] = b
        off_of[nd] = used[b]
        node_new[b * BLK + used[b]] = nd
        loads[b] += deg[nd]
        used[b] += 1
    return blk_of, off_of, node_new, loads


def _plan_core(row_l, col, n_slice, nblk, T):
    spb = T * BLK
    deg = np.bincount(row_l, minlength=n_slice)
    blk_of, off_of, node_new, loads = _balance_blocks(deg, nblk)
    assert loads.max() <= spb, (loads.max(), spb)

    eb = blk_of[row_l]
    eo = off_of[row_l]
    order_e = np.lexsort((eo, eb))
    nslots = nblk * spb
    sl_edge = np.full(nslots, -1, dtype=np.int64)
    sl_off = np.full(nslots, -1, dtype=np.int64)
    sl_col = np.zeros(nslots, dtype=np.int64)
    bstart = np.zeros(nblk + 1, dtype=np.int64)
    np.cumsum(np.bincount(eb, minlength=nblk), out=bstart[1:])
    for b in range(nblk):
        es = order_e[bstart[b]:bstart[b + 1]]
        k = len(es)
        base = b * spb
        sl_edge[base:base + k] = es
        sl_off[base:base + k] = eo[es]
        sl_col[base:base + k] = col[es]
    return dict(node_new=node_new, sl_edge=sl_edge, sl_off=sl_off,
                sl_col=sl_col)


def _wrap_idx16(idx_lin):
    """dma_gather index layout: (128, n/16) int16; slot i at [i%16, i//16],
    replicated to 128 partitions."""
    n = idx_lin.shape[0]
    w = idx_lin.reshape(n // 16, 16).T.astype(np.int16)
    return np.tile(w, (8, 1))


def _prep_inputs(x, M, V, edge_index, ln_g, ln_b,
                 msg_w1, msg_b1, msg_w2, msg_b2,
                 node_w1, node_b1, node_w2, node_b2,
                 inv_w1, inv_b1, inv_w2, inv_b2,
                 gate_w, gate_b):
    n_nodes = x.shape[0]
    n_edges = M.shape[0]
    ns = n_nodes // NC_CORES
    nblk = (ns + BLK - 1) // BLK
    row = edge_index[0].astype(np.int64)
    col = edge_index[1].astype(np.int64)

    Vf = np.ascontiguousarray(V.reshape(n_nodes, AX * W)).astype(np.float32)

    # pass 1: find T (max block load over all cores)
    maxload = 0
    for k in range(NC_CORES):
        m = (row >= k * ns) & (row < (k + 1) * ns)
        deg = np.bincount(row[m] - k * ns, minlength=ns)
        _, _, _, loads = _balance_blocks(deg, nblk)
        maxload = max(maxload, int(loads.max()))
    T = max(2, (maxload + BLK - 1) // BLK)
    spb = T * BLK
    half_split = (nblk + 1) // 2

    core_data = []
    max_uniq = 16
    for k in range(NC_CORES):
        m = (row >= k * ns) & (row < (k + 1) * ns)
        plan = _plan_core(row[m] - k * ns, col[m], ns, nblk, T)
        plan["eidx"] = np.nonzero(m)[0]
        scol = plan["sl_col"]
        u1 = np.unique(scol[:half_split * spb])
        u2 = np.unique(scol[half_split * spb:])
        max_uniq = max(max_uniq, len(u1), len(u2))
        core_data.append(plan)
    tbl_n = ((max_uniq + 127) // 128) * 128
    assert tbl_n <= 32000, tbl_n

    g = ln_g.astype(np.float32)
    beta = ln_b.astype(np.float32)
    consts = dict(
        gamma_rep=np.tile(g[None, :], (BLK, 1)).astype(np.float32),
        ident_bf=np.eye(BLK, dtype=np.float32).astype(BF16),
        iota_row=np.tile(np.arange(BLK, dtype=np.float32)[None, :], (BLK, 1)),
        iota_col=np.arange(BLK, dtype=np.float32)[:, None].copy(),
        ones1_b=np.ones((1, BLK), dtype=np.float32).astype(BF16),
        w1_b=msg_w1.astype(np.float32).astype(BF16),
        c1_col=(beta @ msg_w1 + msg_b1).astype(np.float32)[:, None].copy(),
        w2_b=msg_w2.astype(np.float32).astype(BF16),
        c2row_b=(msg_b2 + beta).astype(np.float32)[None, :].astype(BF16),
        iw1_b=np.concatenate([inv_w1[W:2 * W], inv_w1[2 * W:3 * W],
                              inv_w1[0:W]], axis=0).astype(np.float32)
                              .astype(BF16),
        ib1_col=inv_b1.astype(np.float32)[:, None].copy(),
        iw2_b=inv_w2.astype(np.float32).astype(BF16),
        ib2_col=np.full((BLK, 1), float(np.asarray(inv_b2).ravel()[0]),
                        dtype=np.float32),
        nw1_b=node_w1.astype(np.float32).astype(BF16),
        nb1_col=node_b1.astype(np.float32)[:, None].copy(),
        nw2_b=node_w2.astype(np.float32).astype(BF16),
        nb2row_b=node_b2.astype(np.float32)[None, :].astype(BF16),
        gw_b=gate_w.astype(np.float32).astype(BF16),
        gbrow_b=gate_b.astype(np.float32)[None, :].astype(BF16),
        zero_col=np.zeros((BLK, 1), dtype=np.float32),
        eps_col=np.full((BLK, 1), LN_EPS, dtype=np.float32),
    )

    in_maps = []
    for k in range(NC_CORES):
        p = core_data[k]
        sl_edge, sl_off, sl_col = p["sl_edge"], p["sl_off"], p["sl_col"]
        eidx = p["eidx"]
        node_new = p["node_new"]
        nslots = nblk * spb

        Ms = np.zeros((nslots, D), dtype=np.float32)
        valid = sl_edge >= 0
        Ms[valid] = M[eidx[sl_edge[valid]]]
        Ms = Ms.reshape(nblk, T, BLK, D).transpose(0, 2, 1, 3).copy()

        tabs = np.zeros((2, tbl_n, 64), dtype=np.float32)
        gidx = np.zeros((nblk, BLK, spb // 16), dtype=np.int16)
        for h in range(2):
            lo = 0 if h == 0 else half_split
            hi = half_split if h == 0 else nblk
            cols_h = sl_col[lo * spb:hi * spb]
            uniq, inv = np.unique(cols_h, return_inverse=True)
            tabs[h, :len(uniq), :AX * W] = Vf[uniq]
            inv = inv.reshape(hi - lo, spb)
            for b in range(lo, hi):
                gidx[b] = _wrap_idx16(inv[b - lo])

        offs = sl_off.reshape(nblk, T, BLK)
        idxn_p = offs.transpose(0, 2, 1).astype(np.float32).copy()
        idxn_row = sl_off.reshape(nblk, 1, spb).astype(np.float32).astype(BF16)

        npad = nblk * BLK
        vmask = node_new >= 0
        xp = np.zeros((npad, D), dtype=np.float32)
        xp[vmask] = x[k * ns + node_new[vmask]]
        xT_blk = np.ascontiguousarray(
            xp.reshape(nblk, BLK, D).transpose(0, 2, 1))
        Vn_blk = np.zeros((nblk, BLK, 64), dtype=np.float32)
        Vnp = np.zeros((npad, AX * W), dtype=np.float32)
        Vnp[vmask] = Vf[k * ns + node_new[vmask]]
        Vn_blk[:, :, :AX * W] = Vnp.reshape(nblk, BLK, AX * W)

        im = dict(Ms=Ms, xT_blk=xT_blk, Vn_blk=Vn_blk,
                  tabA=tabs[0].copy(), tabB=tabs[1].copy(), gidx=gidx,
                  idxn_p=idxn_p, idxn_row=idxn_row)
        im.update(consts)
        in_maps.append(im)

    meta = dict(T=T, nblk=nblk, spb=spb, tbl_n=tbl_n, ns=ns,
                half_split=half_split, core_data=core_data,
                n_nodes=n_nodes, n_edges=n_edges)
    return in_maps, meta


# ----------------------------------------------------------------------------
# Device program
# ----------------------------------------------------------------------------

def _build_program(T, nblk, tbl_n, half_split):
    from contextlib import ExitStack

    import concourse.bass as bass  # noqa: F401
    import concourse.tile as tile
    from concourse import bacc, mybir

    f32 = mybir.dt.float32
    bf16 = mybir.dt.bfloat16
    i16 = mybir.dt.int16
    AF = mybir.ActivationFunctionType
    OP = mybir.AluOpType

    spb = T * BLK

    nc = bacc.Bacc("TRN2")

    t_Ms = nc.dram_tensor("Ms", [nblk, BLK, T, D], f32, kind="ExternalInput")
    t_xT = nc.dram_tensor("xT_blk", [nblk, BLK, BLK], f32,
                          kind="ExternalInput")
    t_Vn = nc.dram_tensor("Vn_blk", [nblk, BLK, 64], f32,
                          kind="ExternalInput")
    t_tabA = nc.dram_tensor("tabA", [tbl_n, 64], f32, kind="ExternalInput")
    t_tabB = nc.dram_tensor("tabB", [tbl_n, 64], f32, kind="ExternalInput")
    t_gidx = nc.dram_tensor("gidx", [nblk, BLK, spb // 16], i16,
                            kind="ExternalInput")
    t_idxn_p = nc.dram_tensor("idxn_p", [nblk, BLK, T], f32,
                              kind="ExternalInput")
    t_idxn_row = nc.dram_tensor("idxn_row", [nblk, 1, spb], bf16,
                                kind="ExternalInput")

    cns = {}
    for name, shape, dt in [
        ("gamma_rep", [BLK, D], f32), ("ident_bf", [BLK, BLK], bf16),
        ("iota_row", [BLK, BLK], f32), ("iota_col", [BLK, 1], f32),
        ("ones1_b", [1, BLK], bf16),
        ("w1_b", [D, D], bf16), ("c1_col", [D, 1], f32),
        ("w2_b", [D, D], bf16), ("c2row_b", [1, D], bf16),
        ("iw1_b", [AX * W, D], bf16), ("ib1_col", [D, 1], f32),
        ("iw2_b", [D, 1], bf16), ("ib2_col", [BLK, 1], f32),
        ("nw1_b", [D, D], bf16), ("nb1_col", [D, 1], f32),
        ("nw2_b", [D, D], bf16), ("nb2row_b", [1, D], bf16),
        ("gw_b", [D, W], bf16), ("gbrow_b", [1, W], bf16),
        ("zero_col", [BLK, 1], f32), ("eps_col", [BLK, 1], f32),
    ]:
        cns[name] = nc.dram_tensor(name, shape, dt, kind="ExternalInput")

    t_M2 = nc.dram_tensor("M2s", [nblk, BLK, T, D], f32,
                          kind="ExternalOutput")
    t_x2 = nc.dram_tensor("x2T_out", [nblk, BLK, BLK], f32,
                          kind="ExternalOutput")
    t_V2 = nc.dram_tensor("V2_out", [nblk, BLK, AX * W], f32,
                          kind="ExternalOutput")

    with tile.TileContext(nc) as tc:
        with ExitStack() as ctx:
            cpool = ctx.enter_context(tc.tile_pool(name="consts", bufs=1))
            mpool = ctx.enter_context(tc.tile_pool(name="mblk", bufs=2))
            bpool = ctx.enter_context(tc.tile_pool(name="blockbufs", bufs=2))
            spool = ctx.enter_context(tc.tile_pool(name="smalls", bufs=3))
            tpool = ctx.enter_context(tc.tile_pool(name="tiles", bufs=3))
            ppool = ctx.enter_context(
                tc.tile_pool(name="psum", bufs=6, space="PSUM"))
            mippool = ctx.enter_context(
                tc.tile_pool(name="mip", bufs=2, space="PSUM"))


            c = {}
            for name, tdram in cns.items():
                t = cpool.tile(list(tdram.shape), tdram.dtype, tag=name,
                               name=f"c_{name}")
                nc.sync.dma_start(t[:], tdram[:])
                c[name] = t
            nc.const_aps.aps[(f32, 0.0)] = c["zero_col"][:]
            nc.const_aps.aps[(f32, LN_EPS)] = c["eps_col"][:]

            for b in range(nblk):
                tab = t_tabA if b < half_split else t_tabB

                # ---- node-side prep: V rows + norms for this block
                vn = bpool.tile([BLK, 64], f32, tag="vn", name="vn")
                nc.sync.dma_start(vn[:], t_Vn[b])
                vnsq = spool.tile([BLK, AX * W], f32, tag="vnsq", name="vnsq")
                nc.vector.tensor_tensor(
                    vnsq[:], vn[:, 0:AX * W], vn[:, 0:AX * W], op=OP.mult)
                nsq = spool.tile([BLK, W], f32, tag="nsq", name="nsq")
                nc.vector.tensor_reduce(
                    nsq[:], vnsq[:].rearrange("p (a w) -> p w a", a=AX),
                    axis=mybir.AxisListType.X, op=OP.add)
                nc.scalar.sqrt(vn[:, 48:64], nsq[:])
                vnb = bpool.tile([BLK, 64], bf16, tag="vnb", name="vnb")
                nc.gpsimd.tensor_copy(vnb[:], vn[:])

                # ---- gather col V rows for the whole block
                gx = bpool.tile([BLK, spb // 16], i16, tag="gx", name="gx")
                nc.sync.dma_start(gx[:], t_gidx[b])
                vc = bpool.tile([BLK, T, 64], f32, tag="vc", name="vc")
                nc.gpsimd.dma_gather(vc[:], tab[:], gx[:], spb, spb, 64)

                # ---- M block in + LN stats
                mblk = mpool.tile([BLK, T, D], f32, tag="mblk", name="mblk")
                nc.sync.dma_start(mblk[:], t_Ms[b])
                st = spool.tile([BLK, T, 6], f32, tag="st", name="st")
                for t in range(T):
                    nc.vector.bn_stats(st[:, t, :], mblk[:, t, :])
                # st fields: [cnt_e, mean_e, M2_e, cnt_o, mean_o, M2_o]
                stv = st[:].rearrange("p t (g f) -> p t f g", g=2)
                sm = spool.tile([BLK, T], f32, tag="sm", name="sm")
                nc.vector.tensor_reduce(
                    sm[:], stv[:, :, 1:2, :], axis=mybir.AxisListType.X,
                    op=OP.add)
                m2s_ = spool.tile([BLK, T], f32, tag="m2s_", name="m2s_")
                nc.vector.tensor_reduce(
                    m2s_[:], stv[:, :, 2:3, :], axis=mybir.AxisListType.X,
                    op=OP.add)
                stsq = spool.tile([BLK, T, 2], f32, tag="stsq", name="stsq")
                nc.vector.tensor_tensor(
                    stsq[:].rearrange("p t (o g) -> p t o g", o=1),
                    stv[:, :, 1:2, :], stv[:, :, 1:2, :], op=OP.mult)
                sqs = spool.tile([BLK, T], f32, tag="sqs", name="sqs")
                nc.vector.tensor_reduce(
                    sqs[:], stsq[:], axis=mybir.AxisListType.X, op=OP.add)
                # mean = sm/2 ; var = m2s/128 + sqs/2 - sm^2/4
                a_ = spool.tile([BLK, T], f32, tag="a_", name="a_")
                nc.vector.tensor_tensor(a_[:], sm[:], sm[:], op=OP.mult)
                b2_ = spool.tile([BLK, T], f32, tag="b2_", name="b2_")
                nc.vector.tensor_scalar(b2_[:], sqs[:], 0.5, None, op0=OP.mult)
                var = spool.tile([BLK, T], f32, tag="var", name="var")
                nc.vector.scalar_tensor_tensor(
                    var[:], in0=m2s_[:], scalar=1.0 / D, in1=b2_[:],
                    op0=OP.mult, op1=OP.add)
                var2 = spool.tile([BLK, T], f32, tag="var2", name="var2")
                nc.vector.scalar_tensor_tensor(
                    var2[:], in0=a_[:], scalar=-0.25, in1=var[:],
                    op0=OP.mult, op1=OP.add)
                sd = spool.tile([BLK, T], f32, tag="sd", name="sd")
                nc.scalar.activation(sd[:], var2[:], AF.Sqrt, bias=LN_EPS)
                r_ = spool.tile([BLK, T], f32, tag="r_", name="r_")
                nc.vector.reciprocal(r_[:], sd[:])
                rmu = spool.tile([BLK, T], f32, tag="rmu", name="rmu")
                nc.vector.scalar_tensor_tensor(
                    rmu[:], in0=sm[:], scalar=0.5, in1=r_[:],
                    op0=OP.mult, op1=OP.mult)

                # ---- per-slot node index data
                ixr = bpool.tile([1, spb], bf16, tag="ixr", name="ixr")
                nc.sync.dma_start(ixr[:], t_idxn_row[b])
                ixp = bpool.tile([BLK, T], f32, tag="ixp", name="ixp")
                nc.sync.dma_start(ixp[:], t_idxn_p[b])

                vr = bpool.tile([BLK, T, 64], f32, tag="vr", name="vr")
                ei = bpool.tile([BLK, T, 48], f32, tag="ei", name="ei")
                m2blk = mpool.tile([BLK, T, D], f32, tag="m2blk",
                                   name="m2blk")
                m2blk_b = mpool.tile([BLK, T, D], bf16, tag="m2blk_b",
                                     name="m2blk_b")
                alpha = bpool.tile([BLK, T], f32, tag="alpha", name="alpha")

                # nd = sqrt(sum_a vc^2) into ei[:, :, 0:16]
                vcsq = bpool.tile([BLK, T, AX * W], f32, tag="vcsq",
                                  name="vcsq")
                nc.vector.tensor_tensor(
                    vcsq[:], vc[:, :, 0:AX * W], vc[:, :, 0:AX * W],
                    op=OP.mult)
                nc2t = bpool.tile([BLK, T, W], f32, tag="nc2t", name="nc2t")
                nc.vector.tensor_reduce(
                    nc2t[:], vcsq[:].rearrange("p t (a w) -> p t w a", a=AX),
                    axis=mybir.AxisListType.X, op=OP.add)
                nc.scalar.sqrt(ei[:, :, 0:W], nc2t[:])

                # ---- per-tile: expand row-side V rows via one-hot matmul
                for t in range(T):
                    irp = ppool.tile([BLK, BLK], f32, tag="ps", name="irp")
                    nc.tensor.matmul(
                        irp[:], c["ones1_b"][:],
                        ixr[:, t * BLK:(t + 1) * BLK], start=True, stop=True)
                    stb = tpool.tile([BLK, BLK], bf16, tag="stb", name="stb")
                    nc.vector.tensor_scalar(
                        stb[:], irp[:], c["iota_col"][:, 0:1], None,
                        op0=OP.is_equal)
                    vrp = ppool.tile([BLK, 64], f32, tag="ps", name="vrp")
                    nc.tensor.matmul(vrp[:], stb[:], vnb[:], start=True,
                                     stop=True)
                    nc.scalar.copy(vr[:, t, :], vrp[:])

                # dots / cos at block level
                dotp = bpool.tile([BLK, T, AX * W], f32, tag="dotp",
                                  name="dotp")
                nc.vector.tensor_tensor(
                    dotp[:], vr[:, :, 0:AX * W], vc[:, :, 0:AX * W],
                    op=OP.mult)
                dott = bpool.tile([BLK, T, W], f32, tag="dott", name="dott")
                nc.vector.tensor_reduce(
                    dott[:], dotp[:].rearrange("p t (a w) -> p t w a", a=AX),
                    axis=mybir.AxisListType.X, op=OP.add)
                den = bpool.tile([BLK, T, W], f32, tag="den", name="den")
                nc.vector.tensor_tensor(
                    den[:], vr[:, :, 48:64], ei[:, :, 0:W], op=OP.mult)
                den2 = bpool.tile([BLK, T, W], f32, tag="den2", name="den2")
                nc.vector.tensor_scalar(den2[:], den[:], COS_EPS, None,
                                        op0=OP.add)
                rec = bpool.tile([BLK, T, W], f32, tag="rec", name="rec")
                nc.vector.reciprocal(rec[:], den2[:])
                nc.vector.tensor_tensor(
                    ei[:, :, W:2 * W], dott[:], rec[:], op=OP.mult)
                nc.vector.tensor_copy(ei[:, :, 2 * W:3 * W], vr[:, :, 48:64])
                eib = bpool.tile([BLK, T, 48], bf16, tag="eib", name="eib")
                nc.gpsimd.tensor_copy(eib[:], ei[:])

                mip = mippool.tile([BLK, D], f32, tag="ps", name="mip")

                for t in range(T):
                    # alpha path
                    eitp = ppool.tile([48, BLK], bf16, tag="ps", name="eitp")
                    nc.tensor.transpose(eitp[:], eib[:, t, :],
                                        c["ident_bf"][:])
                    eitb = tpool.tile([48, BLK], bf16, tag="eitb",
                                      name="eitb")
                    nc.scalar.copy(eitb[:], eitp[:])
                    hip = ppool.tile([D, BLK], f32, tag="ps", name="hip")
                    nc.tensor.matmul(hip[:], c["iw1_b"][:], eitb[:],
                                     start=True, stop=True)
                    hib = tpool.tile([D, BLK], bf16, tag="hib", name="hib")
                    nc.scalar.activation(hib[:], hip[:], AF.Relu,
                                         bias=c["ib1_col"][:, 0:1])
                    alp = ppool.tile([BLK, 1], f32, tag="ps", name="alp")
                    nc.tensor.matmul(alp[:], hib[:], c["iw2_b"][:],
                                     start=True, stop=True)
                    nc.scalar.activation(alpha[:, t:t + 1], alp[:],
                                         AF.Sigmoid, bias=c["ib2_col"][:, 0:1])

                    # msg MLP
                    mn = tpool.tile([BLK, D], f32, tag="mn", name="mn")
                    nc.vector.tensor_scalar(
                        mn[:], mblk[:, t, :], r_[:, t:t + 1], rmu[:, t:t + 1],
                        op0=OP.mult, op1=OP.subtract)
                    mng = tpool.tile([BLK, D], f32, tag="mng", name="mng")
                    nc.vector.tensor_tensor(
                        mng[:], mn[:], c["gamma_rep"][:], op=OP.mult)
                    mngb = tpool.tile([BLK, D], bf16, tag="mngb", name="mngb")
                    nc.gpsimd.tensor_copy(mngb[:], mng[:])
                    mntp = ppool.tile([D, BLK], bf16, tag="ps", name="mntp")
                    nc.tensor.transpose(mntp[:], mngb[:], c["ident_bf"][:])
                    mntb = tpool.tile([D, BLK], bf16, tag="mntb", name="mntb")
                    nc.scalar.copy(mntb[:], mntp[:])
                    h1p = ppool.tile([D, BLK], f32, tag="ps", name="h1p")
                    nc.tensor.matmul(h1p[:], c["w1_b"][:], mntb[:],
                                     start=True, stop=True)
                    h1b = tpool.tile([D, BLK], bf16, tag="h1b", name="h1b")
                    nc.scalar.activation(h1b[:], h1p[:], AF.Relu,
                                         bias=c["c1_col"][:, 0:1])
                    h2p = ppool.tile([BLK, D], f32, tag="ps", name="h2p")
                    nc.tensor.matmul(h2p[:], h1b[:], c["w2_b"][:],
                                     start=True, stop=False)
                    nc.tensor.matmul(h2p[:], c["ones1_b"][:], c["c2row_b"][:],
                                     start=False, stop=True)
                    nc.vector.tensor_tensor(
                        m2blk[:, t, :], h2p[:], mng[:], op=OP.add)
                    nc.gpsimd.tensor_copy(m2blk_b[:, t, :], m2blk[:, t, :])

                    # scatter via alpha-scaled one-hot
                    sab = tpool.tile([BLK, BLK], bf16, tag="sab", name="sab")
                    nc.vector.tensor_scalar(
                        sab[:], c["iota_row"][:], ixp[:, t:t + 1],
                        alpha[:, t:t + 1], op0=OP.is_equal, op1=OP.mult)
                    nc.tensor.matmul(mip[:], sab[:], m2blk_b[:, t, :],
                                     start=(t == 0), stop=(t == T - 1))

                nc.sync.dma_start(t_M2[b], m2blk[:])

                # ---- node phase
                mib = spool.tile([BLK, D], bf16, tag="mib", name="mib")
                nc.scalar.copy(mib[:], mip[:])
                mitp = ppool.tile([D, BLK], bf16, tag="ps", name="mitp")
                nc.tensor.transpose(mitp[:], mib[:], c["ident_bf"][:])
                mitb = spool.tile([D, BLK], bf16, tag="mitb", name="mitb")
                nc.scalar.copy(mitb[:], mitp[:])
                hnp = ppool.tile([D, BLK], f32, tag="ps", name="hnp")
                nc.tensor.matmul(hnp[:], c["nw1_b"][:], mitb[:], start=True,
                                 stop=True)
                hnb = spool.tile([D, BLK], bf16, tag="hnb", name="hnb")
                nc.scalar.activation(hnb[:], hnp[:], AF.Relu,
                                     bias=c["nb1_col"][:, 0:1])
                x2p_ = ppool.tile([D, BLK], f32, tag="ps", name="x2p_")
                nc.tensor.matmul(x2p_[:], c["nw2_b"][:], hnb[:], start=True,
                                 stop=False)
                nc.tensor.matmul(x2p_[:], c["nb2row_b"][:], c["ones1_b"][:],
                                 start=False, stop=True)
                xt = spool.tile([BLK, BLK], f32, tag="xt", name="xt")
                nc.sync.dma_start(xt[:], t_xT[b])
                x2t = spool.tile([BLK, BLK], f32, tag="x2t", name="x2t")
                nc.vector.tensor_tensor(x2t[:], x2p_[:], xt[:], op=OP.add)
                nc.sync.dma_start(t_x2[b], x2t[:])
                x2tb = spool.tile([BLK, BLK], bf16, tag="x2tb", name="x2tb")
                nc.gpsimd.tensor_copy(x2tb[:], x2t[:])
                gp_ = ppool.tile([BLK, W], f32, tag="ps", name="gp_")
                nc.tensor.matmul(gp_[:], x2tb[:], c["gw_b"][:], start=True,
                                 stop=False)
                nc.tensor.matmul(gp_[:], c["ones1_b"][:], c["gbrow_b"][:],
                                 start=False, stop=True)
                gates = spool.tile([BLK, W], f32, tag="gates", name="gates")
                nc.scalar.activation(gates[:], gp_[:], AF.Sigmoid)
                v2t = spool.tile([BLK, AX * W], f32, tag="v2t", name="v2t")
                nc.vector.tensor_tensor(
                    v2t[:].rearrange("p (a w) -> p a w", a=AX),
                    vn[:, 0:AX * W].rearrange("p (a w) -> p a w", a=AX),
                    gates[:].rearrange("p (o w) -> p o w", o=1)
                        .to_broadcast([BLK, AX, W]),
                    op=OP.mult)
                nc.sync.dma_start(t_V2[b], v2t[:])

    nc.compile()
    return nc


def _get_program(T, nblk, tbl_n, half_split):
    key = (T, nblk, tbl_n, half_split)
    if key not in _PROGRAM_CACHE:
        _PROGRAM_CACHE[key] = _build_program(T, nblk, tbl_n, half_split)
    return _PROGRAM_CACHE[key]


# ----------------------------------------------------------------------------
# Entry point
# ----------------------------------------------------------------------------

def kernel(**inputs):
    global LAST_RESULTS, LAST_EXEC_NS
    np_inputs = {k: np.asarray(v) for k, v in inputs.items()}
    in_maps, meta = _prep_inputs(**np_inputs)
    T, nblk, spb = meta["T"], meta["nblk"], meta["spb"]
    ns = meta["ns"]
    n_nodes, n_edges = meta["n_nodes"], meta["n_edges"]

    nc = _get_program(T, nblk, meta["tbl_n"], meta["half_split"])

    from concourse.bass_utils import run_bass_kernel_spmd
    trace = bool(int(os.environ.get("KERNEL_TRACE", "0")))
    res = run_bass_kernel_spmd(nc, in_maps, list(range(NC_CORES)),
                               trace=trace)
    LAST_RESULTS = res
    LAST_EXEC_NS = res.exec_time_ns

    x2 = np.empty((n_nodes, D), dtype=np.float32)
    M2 = np.empty((n_edges, D), dtype=np.float32)
    V2 = np.empty((n_nodes, AX, W), dtype=np.float32)
    for k in range(NC_CORES):
        out = res.results[k]
        p = meta["core_data"][k]
        node_new, sl_edge, eidx = p["node_new"], p["sl_edge"], p["eidx"]
        vmask = node_new >= 0
        gnode = k * ns + node_new[vmask]
        x2p = out["x2T_out"].transpose(0, 2, 1).reshape(nblk * BLK, D)
        x2[gnode] = x2p[vmask]
        v2p = out["V2_out"].reshape(nblk * BLK, AX * W)
        V2[gnode] = v2p[vmask].reshape(-1, AX, W)
        m2s = out["M2s"].transpose(0, 2, 1, 3).reshape(nblk * spb, D)
        emask = sl_edge >= 0
        M2[eidx[sl_edge[emask]]] = m2s[emask]
    return (x2, M2, V2)
